# revision 1
# baseline (speedup 1.0000x reference)
"""Trainium2 Bass kernel for nn_AttentionLayer (sparse_attention).

Math per batch b (8 batches -> 8 cores, data parallel):
  q = Wq @ x, k = Wk @ x, v = Wv @ x            (x as [C=768, HW=4096])
  Qf/Kf/Vf = [L=6144, W=64]   (row index l = c*64 + h)
  S = Qf @ Kf^T  [6144, 6144]; beta = softmax(S, axis=-1)
  attn = beta @ Vf; out = gamma * Wc @ attn(as [96,4096]) + x

Kernel strategy (per core):
  - Reorder both the query index and the key/value index as l' = h*96 + c
    (softmax over l is permutation invariant; queries are independent) so all
    on-chip layout changes become contiguous copies / 128-col PE transposes.
  - Compute Q^w, K^w, v^w in [w=64 part, l'=6144 free] layout with per-h
    matmuls: out_h[w, c] = x[:, h*64:(h+1)*64].T @ [Wq^T|Wk^T|Wv^T].
  - S^T blocks: [l' part, ch' free] = K^w-chunk.T @ Q^w  (contraction over w).
  - exp on ScalarE (no max subtraction: |logits| <~ 20 for these inputs, exp
    stays well inside fp32 range), bf16 output.
  - PV: U[w(+denom), ch'] accumulated in PSUM with vf as the stationary
    operand; 65th lhsT column of ones accumulates the softmax denominator.
  - Per h: PE-transpose U[:, h*96:(h+1)*96] -> [96, 65]; divide by denom via
    per-partition reciprocal + tensor_scalar; gives attn in [c, h*64+w] layout.
  - Final projection with host-side fused (gamma*Wc)^T, residual add, store.
  - All matmul operands are bf16 (PE runs fp32 at 1/4 rate); accumulation,
    softmax denominators and the residual path stay fp32.
"""

import os
from contextlib import ExitStack

import ml_dtypes
import numpy as np

import concourse.bass as bass
import concourse.mybir as mybir
import concourse.tile as tile
from concourse import bacc
from concourse.bass_utils import run_bass_kernel_spmd
from concourse.masks import make_identity

C = 768
CK = 96
H = 64
W = 64
HW = H * W          # 4096
L = CK * H          # 6144
NCHUNK = L // 128   # 48
FP = mybir.dt.float32
BF = mybir.dt.bfloat16

_CACHE = {}


def _build_program():
    nc = bacc.Bacc(
        "TRN2",
        target_bir_lowering=False,
        debug=False,
        enable_asserts=False,
        num_devices=8,
    )
    x = nc.dram_tensor("x", [C, HW], FP, kind="ExternalInput").ap()
    wt = nc.dram_tensor("wt", [C, 3 * CK], BF, kind="ExternalInput").ap()
    wct = nc.dram_tensor("wct", [CK, C], BF, kind="ExternalInput").ap()
    out = nc.dram_tensor("out", [C, HW], FP, kind="ExternalOutput").ap()

    x_t = x.rearrange("(k p) f -> k p f", p=128)      # [6, 128, 4096]
    out_t = out.rearrange("(k p) f -> k p f", p=128)  # [6, 128, 4096]

    with ExitStack() as ctx:
        tc = ctx.enter_context(tile.TileContext(nc))
        singles = ctx.enter_context(tc.tile_pool(name="singles", bufs=1))
        big = ctx.enter_context(tc.tile_pool(name="big", bufs=1))
        xpool = ctx.enter_context(tc.tile_pool(name="xpool", bufs=2))
        xbp = ctx.enter_context(tc.tile_pool(name="xbp", bufs=2))
        expp = ctx.enter_context(tc.tile_pool(name="expp", bufs=3))
        outp = ctx.enter_context(tc.tile_pool(name="outp", bufs=2))
        rp = ctx.enter_context(tc.tile_pool(name="rp", bufs=2))
        ps_a = ctx.enter_context(tc.tile_pool(name="ps_a", bufs=2, space="PSUM"))
        ps_s = ctx.enter_context(tc.tile_pool(name="ps_s", bufs=2, space="PSUM"))
        ps_u = ctx.enter_context(tc.tile_pool(name="ps_u", bufs=1, space="PSUM"))

        # ---- constants ----
        ident = singles.tile([65, 65], FP)
        make_identity(nc, ident)
        identb = singles.tile([65, 65], BF)
        make_identity(nc, identb)
        wt_sb = singles.tile([128, 6, 3 * CK], BF)
        nc.sync.dma_start(out=wt_sb, in_=wt.rearrange("(k p) n -> p k n", p=128))
        wct_sb = singles.tile([CK, C], BF)
        nc.sync.dma_start(out=wct_sb, in_=wct)

        # Stage-1 outputs are split into 6 column-tiles of 1024 (= 8 l'-chunks)
        # each, so attention matmuls unblock as soon as their slice is ready
        # (Tile dependency tracking is per-tile).
        # q, k and v^w share one tile per column-group (q = cols 0:1024,
        # k = 1024:2048, v^w = 2048:3072) so each h-block's evacuation is a
        # single 3-run strided copy.
        kv_tiles = [
            big.tile([64, 3072], BF, tag=f"kv{t}", name=f"kv{t}") for t in range(6)
        ]
        q_tiles = [kv_tiles[t][:, 0:1024] for t in range(6)]
        vf_tiles = [
            singles.tile([128, 8, 65], BF, tag=f"vf{t}", name=f"vf{t}")
            for t in range(6)
        ]
        for t in range(6):
            nc.vector.memset(vf_tiles[t], 1.0)  # col 64 of each chunk = 1.0

        def evac(dst_tiles, h, src, engine):
            """Copy [64, 96] h-block into the split 1024-col tiles."""
            p0 = (h * CK) // 1024
            off = h * CK - p0 * 1024
            copy = (
                (lambda o, i: nc.scalar.copy(o, i))
                if engine == "act"
                else (lambda o, i: nc.vector.tensor_copy(out=o, in_=i))
            )
            if off + CK <= 1024:
                copy(dst_tiles[p0][:, off:off + CK], src)
            else:
                f1 = 1024 - off
                copy(dst_tiles[p0][:, off:1024], src[:, 0:f1])
                copy(dst_tiles[p0 + 1][:, 0:CK - f1], src[:, f1:CK])

        # ---- stage 1: QKV in w-major layout ----
        for hb in range(8):  # h in batches of 8
            xc = xpool.tile([128, 6, 512], FP, tag="xc")
            nc.sync.dma_start(
                out=xc,
                in_=x_t[:, :, hb * 512:(hb + 1) * 512].rearrange("k p f -> p k f"),
            )
            xb = xbp.tile([128, 6, 512], BF, tag="xb")
            nc.gpsimd.tensor_copy(out=xb, in_=xc)  # fp32 -> bf16 cast
            for hp in range(4):
                # Two h-blocks per matmul: lhsT spans 128 x-columns, so the
                # output uses all 128 PE partitions (rows 0:64 = even h,
                # 64:128 = odd h) — halves the QKV matmul count.
                qp = ps_a.tile([128, 3 * CK], FP, tag="mm1")
                for kc in range(6):
                    nc.tensor.matmul(
                        qp,
                        xb[:, kc, hp * 128:(hp + 1) * 128],
                        wt_sb[:, kc, :],
                        start=(kc == 0),
                        stop=(kc == 5),
                    )
                for sub in range(2):
                    h = hb * 8 + hp * 2 + sub
                    qs = qp[sub * 64:(sub + 1) * 64, :]
                    p0 = (h * CK) // 1024
                    off = h * CK - p0 * 1024
                    if off + CK <= 1024:
                        # merged q+k+v evacuation: [64, 3, 96] -> three
                        # 1024-strided runs of the shared tile, one instruction
                        nc.vector.tensor_copy(
                            out=kv_tiles[p0].rearrange(
                                "p (three c) -> p three c", three=3
                            )[:, :, off:off + CK],
                            in_=qs.rearrange("p (three c) -> p three c", three=3),
                        )
                    else:
                        evac(q_tiles, h, qs[:, 0:CK], "dve")
                        kvk = [kv_tiles[t][:, 1024:2048] for t in range(6)]
                        kvv = [kv_tiles[t][:, 2048:3072] for t in range(6)]
                        evac(kvk, h, qs[:, CK:2 * CK], "dve")
                        evac(kvv, h, qs[:, 2 * CK:3 * CK], "dve")
                # vf chunks via PE transpose, batched once a full 1024-col vw
                # tile is complete (avoids WAR ping-pong with later evacs).
                h = hb * 8 + hp * 2 + 1
                t_done = ((h + 1) * CK) // 1024
                t_prev = ((h - 1) * CK) // 1024
                for t in range(t_prev, t_done):
                    for c8 in range(8):
                        lt = t * 8 + c8
                        tp = ps_a.tile([128, 64], BF, tag="mm1")
                        nc.tensor.transpose(
                            tp,
                            kv_tiles[t][:, 2048 + c8 * 128:2048 + (c8 + 1) * 128],
                            identb[0:64, 0:64],
                        )
                        nc.vector.tensor_copy(out=vf_tiles[t][:, c8, 0:64], in_=tp)

        # ---- stage 2: attention (S^T, exp, PV accumulate) ----
        # un is split per pass so stage-3 transposes unblock as passes finish
        # (Tile dependency tracking is per-tile).
        un_tiles = [
            big.tile([65, 1024], FP, tag=f"un{p}", name=f"un{p}") for p in range(6)
        ]
        for p in range(6):  # ch' passes of 1024
            up = ps_u.tile([65, 1024], FP, tag="u")
            for lt in range(NCHUNK):
                sp = ps_s.tile([128, 1024], FP, tag="s")
                for j in range(2):
                    nc.tensor.matmul(
                        sp[:, j * 512:(j + 1) * 512],
                        kv_tiles[lt // 8][:, 1024 + (lt % 8) * 128:1024 + (lt % 8 + 1) * 128],
                        q_tiles[p][:, j * 512:(j + 1) * 512],
                        start=True,
                        stop=True,
                    )
                es = expp.tile([128, 1024], BF, tag="es")
                nc.scalar.activation(es, sp, mybir.ActivationFunctionType.Exp)
                for j in range(2):
                    nc.tensor.matmul(
                        up[:, j * 512:(j + 1) * 512],
                        vf_tiles[lt // 8][:, lt % 8, :],
                        es[:, j * 512:(j + 1) * 512],
                        start=(lt == 0),
                        stop=(lt == NCHUNK - 1),
                    )
            nc.vector.tensor_copy(out=un_tiles[p], in_=up)

        # ---- stage 3+4 interleaved, jj-major: normalize 8 h-blocks, then
        # immediately project + residual + store that 512-column chunk, so
        # pool-slot allocation order matches data readiness and the tail
        # shrinks to the last pass's chunks. ----
        attn_tiles = [
            big.tile([CK, 512], BF, tag=f"attn{jj}", name=f"attn{jj}")
            for jj in range(8)
        ]
        for jj in range(8):
            for hl in range(8):
                h = jj * 8 + hl
                p0 = (h * CK) // 1024
                off = h * CK - p0 * 1024
                if off + CK <= 1024:
                    src = un_tiles[p0][:, off:off + CK]
                else:
                    f1 = 1024 - off
                    st = rp.tile([65, CK], FP, tag="st")
                    nc.vector.tensor_copy(
                        out=st[:, 0:f1], in_=un_tiles[p0][:, off:1024]
                    )
                    nc.vector.tensor_copy(
                        out=st[:, f1:CK], in_=un_tiles[p0 + 1][:, 0:CK - f1]
                    )
                    src = st
                tp = ps_a.tile([CK, 65], FP, tag="mm1")
                nc.tensor.transpose(tp, src, ident)
                r = rp.tile([CK, 1], FP, tag="r")
                nc.vector.reciprocal(r, tp[:, 64:65])
                nc.vector.tensor_scalar_mul(
                    attn_tiles[jj][:, hl * 64:(hl + 1) * 64], tp[:, 0:64], r
                )
            xr = xpool.tile([128, 6, 512], FP, tag="xc")
            nc.sync.dma_start(
                out=xr,
                in_=x_t[:, :, jj * 512:(jj + 1) * 512].rearrange("k p f -> p k f"),
            )
            ob = outp.tile([128, 6, 512], FP, tag="ob")
            for co in range(6):
                op = ps_a.tile([128, 512], FP, tag="mm1")
                nc.tensor.matmul(
                    op,
                    wct_sb[:, co * 128:(co + 1) * 128],
                    attn_tiles[jj],
                    start=True,
                    stop=True,
                )
                nc.vector.tensor_add(ob[:, co, :], op, xr[:, co, :])
            nc.sync.dma_start(
                out=out_t[:, :, jj * 512:(jj + 1) * 512].rearrange("k p f -> p k f"),
                in_=ob,
            )

    nc.finalize()
    return nc


def _get_program():
    if "nc" not in _CACHE:
        _CACHE["nc"] = _build_program()
    return _CACHE["nc"]


def _host_weights(Wq, Wk, Wv, Wc, gamma):
    wt_host = np.ascontiguousarray(
        np.concatenate([Wq.T, Wk.T, Wv.T], axis=1)
    ).astype(ml_dtypes.bfloat16)                       # [768, 288]
    wct_host = np.ascontiguousarray((gamma[0] * Wc).T).astype(
        ml_dtypes.bfloat16
    )                                                  # [96, 768]
    return wt_host, wct_host


def kernel(x, Wq, Wk, Wv, Wc, gamma):
    x = np.asarray(x, dtype=np.float32)
    Wq = np.asarray(Wq, dtype=np.float32)
    Wk = np.asarray(Wk, dtype=np.float32)
    Wv = np.asarray(Wv, dtype=np.float32)
    Wc = np.asarray(Wc, dtype=np.float32)
    gamma = np.asarray(gamma, dtype=np.float32)

    B = x.shape[0]
    assert x.shape == (B, C, H, W) and B == 8

    wt_host, wct_host = _host_weights(Wq, Wk, Wv, Wc, gamma)
    in_maps = [
        {
            "x": np.ascontiguousarray(x[b].reshape(C, HW)),
            "wt": wt_host,
            "wct": wct_host,
        }
        for b in range(B)
    ]

    nc = _get_program()
    trace = os.environ.get("KERNEL_TRACE", "0") == "1"
    res = run_bass_kernel_spmd(
        nc, in_maps, core_ids=list(range(8)), trace=trace
    )
    if trace and res.exec_time_ns is not None:
        print(f"HW exec time: {res.exec_time_ns} ns")
        _CACHE["exec_time_ns"] = res.exec_time_ns

    out = np.stack([r["out"].reshape(C, H, W) for r in res.results])
    return out

